# revision 10
# baseline (speedup 1.0000x reference)
"""GQA attention kernel for Trainium2, 8 NeuronCores.

Sharding: core = b*4 + g  (b = batch 0..1, g = kv-head group 0..3).
Each core handles one batch and one kv group (1 kv head + its 4 query heads).
wq/wo are split by head group (column/row), wk/wv by kv head. The output
projection partial sums (one per group) are reduced on the host.

Everything on-device runs in "transposed activation" space: activations are
[feature, seq] so every matmul contraction dim lands on SBUF partitions.
  Q^T[h] = wq_h^T @ x^T          (lhsT = wq tile, rhs = x^T tile)
  S^T    = K_roped^T.T @ Q_roped^T   -> [keys, queries]
  softmax: no max-subtraction (scores are O(10); exp cannot overflow);
           denominator via ones-vector matmul over the partition axis
  O^T    = V.T @ P^T  accumulated over key tiles
  y^T    = wo^T @ (O^T / D)
RoPE in [d, s] layout: rope(Z) = Z*C + (Pswap @ Z)*Sg, with the pair-swap
permutation done on the PE and the sign folded into the host-built Sg tile.

Matmul operands are float32r (single-pass PE at full rate for free dims
>= 256); the BIR verifier requires producers of f32r-matmul operands to
emit float32r, so those tiles are typed f32r end to end.
"""

import sys

sys.path.insert(0, "/opt/trn_rl_repo")

from contextlib import ExitStack

import numpy as np

import concourse.bass as bass
import concourse.tile as tile
from concourse import bacc, mybir
from concourse import bass_utils

F32 = mybir.dt.float32
F32R = mybir.dt.float32r
MULT = mybir.AluOpType.mult
EXP = mybir.ActivationFunctionType.Exp

S = 2048          # sequence length
DM = 2048         # d_model
DH = 128          # head dim
HPC = 4           # query heads per core (= n_rep; one kv group per core)
N_CORES = 8
CH = 512          # query-chunk width (and s-chunk width)
NCHUNK = S // CH  # 4
NT = DM // 128    # 16 contraction tiles of d_model
SCALE = 1.0 / float(np.sqrt(DH))

_CACHE = {}


def _build():
    nc = bacc.Bacc("TRN2", target_bir_lowering=False, debug=False)

    xT = nc.dram_tensor("xT", [DM, S], F32R, kind="ExternalInput").ap()
    wq = nc.dram_tensor("wq", [DM, HPC * DH], F32R, kind="ExternalInput").ap()
    wk = nc.dram_tensor("wk", [DM, DH], F32R, kind="ExternalInput").ap()
    wv = nc.dram_tensor("wv", [DM, DH], F32R, kind="ExternalInput").ap()
    wo = nc.dram_tensor("wo", [HPC * DH, DM], F32R, kind="ExternalInput").ap()
    cs = nc.dram_tensor("cs", [DH, S], F32, kind="ExternalInput").ap()
    sn = nc.dram_tensor("sn", [DH, S], F32, kind="ExternalInput").ap()
    pswap = nc.dram_tensor("pswap", [DH, DH], F32R, kind="ExternalInput").ap()
    ident = nc.dram_tensor("ident", [128, 128], F32, kind="ExternalInput").ap()
    masks = nc.dram_tensor("masks", [128, 4 * CH], F32, kind="ExternalInput").ap()
    ones = nc.dram_tensor("ones", [128, 1], F32R, kind="ExternalInput").ap()
    yT = nc.dram_tensor("yT", [DM, S], F32, kind="ExternalOutput").ap()

    with tile.TileContext(nc) as tc, ExitStack() as ctx:
        consts = ctx.enter_context(tc.tile_pool(name="consts", bufs=1))
        wpool = ctx.enter_context(tc.tile_pool(name="wpool", bufs=1))
        persist = ctx.enter_context(tc.tile_pool(name="persist", bufs=1))
        xpool = ctx.enter_context(tc.tile_pool(name="xpool", bufs=6))
        qpool = ctx.enter_context(tc.tile_pool(name="qpool", bufs=1))
        rtmp = ctx.enter_context(tc.tile_pool(name="rtmp", bufs=2))
        ppool = ctx.enter_context(tc.tile_pool(name="ppool", bufs=4))
        otpool = ctx.enter_context(tc.tile_pool(name="otpool", bufs=1))
        misc = ctx.enter_context(tc.tile_pool(name="misc", bufs=2))
        ytpool = ctx.enter_context(tc.tile_pool(name="ytpool", bufs=3))
        # PSUM: 8 banks total. One tag per pool so bufs == banks.
        ps_acc = ctx.enter_context(tc.tile_pool(name="ps_acc", bufs=4, space="PSUM"))
        ps_st = ctx.enter_context(tc.tile_pool(name="ps_st", bufs=2, space="PSUM"))
        ps_d = ctx.enter_context(tc.tile_pool(name="ps_d", bufs=2, space="PSUM"))

        # ---------------- constants & weights ----------------
        cs_sb = consts.tile([DH, S], F32, tag="cs")
        nc.sync.dma_start(cs_sb, cs)
        sn_sb = consts.tile([DH, S], F32, tag="sn")
        nc.sync.dma_start(sn_sb, sn)
        pswap_sb = consts.tile([DH, DH], F32R, tag="pswap")
        nc.sync.dma_start(pswap_sb, pswap)
        ident_sb = consts.tile([128, 128], F32, tag="ident")
        nc.sync.dma_start(ident_sb, ident)
        masks_sb = consts.tile([128, 4, CH], F32, tag="masks")
        nc.sync.dma_start(masks_sb, masks.rearrange("p (t n) -> p t n", t=4))
        ones_col = consts.tile([128, 1], F32R, tag="ones")
        nc.sync.dma_start(ones_col, ones)

        wq_sb = wpool.tile([128, NT, HPC * DH], F32R, tag="wq")
        nc.sync.dma_start(wq_sb, wq.rearrange("(t p) n -> p t n", p=128))
        wk_sb = wpool.tile([128, NT, DH], F32R, tag="wk")
        nc.sync.dma_start(wk_sb, wk.rearrange("(t p) n -> p t n", p=128))
        wv_sb = wpool.tile([128, NT, DH], F32R, tag="wv")
        nc.sync.dma_start(wv_sb, wv.rearrange("(t p) n -> p t n", p=128))
        wo_sb = wpool.tile([128, HPC, DM], F32R, tag="wo")
        nc.sync.dma_start(wo_sb, wo.rearrange("(h p) n -> p h n", p=128))

        kt_sb = persist.tile([DH, S], F32R, tag="kt")       # roped K^T
        v_sb = persist.tile([128, S // 128, DH], F32R, tag="v")  # V in [s, d]

        def rope(raw_ps, c, out_ap):
            """out = raw*C + (Pswap @ raw)*Sg for s-chunk c; raw_ps is PSUM."""
            col = c * CH
            raw_sb = misc.tile([128, CH], F32R, tag="raw")
            nc.scalar.copy(raw_sb, raw_ps)
            sw_ps = ps_st.tile([128, CH], F32, tag="st")
            nc.tensor.matmul(sw_ps, pswap_sb, raw_sb, start=True, stop=True)
            ta = rtmp.tile([128, CH], F32, tag="ra")
            nc.gpsimd.tensor_tensor(ta, raw_sb, cs_sb[:, col:col + CH], MULT)
            tb = rtmp.tile([128, CH], F32, tag="rb")
            nc.vector.tensor_tensor(tb, sw_ps, sn_sb[:, col:col + CH], MULT)
            nc.vector.tensor_add(out_ap, ta, tb)

        for c in range(NCHUNK):
            col = c * CH

            # ------------ Q projection (pass A over x^T tiles) ------------
            acc_q = [ps_acc.tile([128, CH], F32, tag="acc", name=f"accq{c}_{i}")
                     for i in range(HPC)]
            for t in range(NT):
                xt = xpool.tile([128, CH], F32R, tag="xt")
                nc.sync.dma_start(xt, xT[t * 128:(t + 1) * 128, col:col + CH])
                for h in range(HPC):
                    nc.tensor.matmul(
                        acc_q[h],
                        wq_sb[:, t, h * DH:(h + 1) * DH],
                        xt,
                        start=(t == 0),
                        stop=(t == NT - 1),
                    )
            qt_sb = qpool.tile([128, HPC, CH], F32R, tag="qt")
            for h in range(HPC):
                rope(acc_q[h], c, qt_sb[:, h, :])

            # ------------ K,V projections (pass B over x^T tiles) ------------
            acc_k = ps_acc.tile([128, CH], F32, tag="acc")
            acc_v = ps_acc.tile([128, CH], F32, tag="acc")
            for t in range(NT):
                xt = xpool.tile([128, CH], F32R, tag="xt")
                nc.sync.dma_start(xt, xT[t * 128:(t + 1) * 128, col:col + CH])
                nc.tensor.matmul(
                    acc_k, wk_sb[:, t, :], xt,
                    start=(t == 0), stop=(t == NT - 1),
                )
                nc.tensor.matmul(
                    acc_v, wv_sb[:, t, :], xt,
                    start=(t == 0), stop=(t == NT - 1),
                )
            rope(acc_k, c, kt_sb[:, col:col + CH])
            vt_sb = misc.tile([128, CH], F32, tag="vt")
            nc.scalar.copy(vt_sb, acc_v)
            for u in range(CH // 128):
                tp_ps = ps_st.tile([128, 128], F32, tag="st")
                nc.tensor.transpose(tp_ps, vt_sb[:, u * 128:(u + 1) * 128], ident_sb)
                nc.vector.tensor_copy(v_sb[:, 4 * c + u, :], tp_ps)

            # ------------ attention for query chunk c ------------
            ot_sb = otpool.tile([128, HPC, CH], F32R, tag="ot")
            njt = 4 * c + 4
            for h in range(HPC):
                ot_ps = ps_acc.tile([128, CH], F32, tag="acc")
                d_ps = ps_d.tile([1, CH], F32, tag="d")
                for j in range(njt):
                    st_ps = ps_st.tile([128, CH], F32, tag="st")
                    nc.tensor.matmul(
                        st_ps,
                        kt_sb[:, j * 128:(j + 1) * 128],
                        qt_sb[:, h, :],
                        start=True, stop=True,
                    )
                    p = ppool.tile([128, CH], F32R, tag="p")
                    nc.scalar.activation(p, st_ps, EXP, scale=SCALE)
                    t = j - 4 * c
                    if t >= 0:
                        nc.vector.tensor_mul(p, p, masks_sb[:, t, :])
                    nc.tensor.matmul(
                        d_ps, ones_col, p,
                        start=(j == 0), stop=(j == njt - 1),
                        skip_group_check=True,
                    )
                    nc.tensor.matmul(
                        ot_ps, v_sb[:, j, :], p,
                        start=(j == 0), stop=(j == njt - 1),
                        skip_group_check=True,
                    )
                rd = misc.tile([1, CH], F32, tag="rd")
                nc.vector.reciprocal(rd, d_ps)
                bc_sb = misc.tile([128, CH], F32, tag="bcsb")
                nc.gpsimd.partition_broadcast(bc_sb, rd)
                nc.vector.tensor_mul(ot_sb[:, h, :], ot_ps, bc_sb)

            # ------------ output projection for chunk c ------------
            for dt_ in range(NT):
                yt_ps = ps_acc.tile([128, CH], F32, tag="acc")
                for h in range(HPC):
                    nc.tensor.matmul(
                        yt_ps,
                        wo_sb[:, h, dt_ * 128:(dt_ + 1) * 128],
                        ot_sb[:, h, :],
                        start=(h == 0), stop=(h == HPC - 1),
                    )
                yt_sb = ytpool.tile([128, CH], F32, tag="ytsb")
                nc.any.tensor_copy(yt_sb, yt_ps)
                nc.sync.dma_start(yT[dt_ * 128:(dt_ + 1) * 128, col:col + CH], yt_sb)

    nc.compile()
    return nc


def _host_prep(x, freqs_cos, freqs_sin, wq, wk, wv, wo):
    """Build the 8 per-core input maps."""
    cos_t = np.ascontiguousarray(freqs_cos.T)  # [64, S]
    sin_t = np.ascontiguousarray(freqs_sin.T)
    cs = np.repeat(cos_t, 2, axis=0).astype(np.float32)        # [128, S]
    sn = np.repeat(sin_t, 2, axis=0).astype(np.float32)
    sn[0::2] *= -1.0

    pswap = np.zeros((DH, DH), dtype=np.float32)
    idx = np.arange(0, DH, 2)
    pswap[idx, idx + 1] = 1.0
    pswap[idx + 1, idx] = 1.0

    ident = np.eye(128, dtype=np.float32)

    # masks[t][jj, ii] = 1 where query ii >= key (128*t + jj), for the 4
    # diagonal key tiles of each 512-wide query chunk.
    ii = np.arange(CH)[None, :]
    jj = np.arange(128)[:, None]
    masks = np.concatenate(
        [(ii >= 128 * t + jj).astype(np.float32) for t in range(4)], axis=1
    )  # [128, 4*CH]

    xTs = [np.ascontiguousarray(x[b].T).astype(np.float32) for b in range(2)]

    in_maps = []
    for core in range(N_CORES):
        b, g = divmod(core, HPC)
        in_maps.append({
            "xT": xTs[b],
            "wq": np.ascontiguousarray(wq[:, g * HPC * DH:(g + 1) * HPC * DH]),
            "wk": np.ascontiguousarray(wk[:, g * DH:(g + 1) * DH]),
            "wv": np.ascontiguousarray(wv[:, g * DH:(g + 1) * DH]),
            "wo": np.ascontiguousarray(wo[g * HPC * DH:(g + 1) * HPC * DH, :]),
            "cs": cs, "sn": sn, "pswap": pswap, "ident": ident, "masks": masks,
            "ones": np.ones((128, 1), dtype=np.float32),
        })
    return in_maps


def kernel(x, freqs_cos, freqs_sin, mask, wq, wk, wv, wo):
    x = np.asarray(x, dtype=np.float32)
    freqs_cos = np.asarray(freqs_cos, dtype=np.float32)
    freqs_sin = np.asarray(freqs_sin, dtype=np.float32)
    wq = np.asarray(wq, dtype=np.float32)
    wk = np.asarray(wk, dtype=np.float32)
    wv = np.asarray(wv, dtype=np.float32)
    wo = np.asarray(wo, dtype=np.float32)

    if "nc" not in _CACHE:
        _CACHE["nc"] = _build()
    nc = _CACHE["nc"]

    in_maps = _host_prep(x, freqs_cos, freqs_sin, wq, wk, wv, wo)
    res = bass_utils.run_bass_kernel_spmd(nc, in_maps, core_ids=list(range(N_CORES)))

    out = np.empty((2, S, DM), dtype=np.float32)
    for b in range(2):
        acc = res.results[b * HPC]["yT"].astype(np.float32)
        for g in range(1, HPC):
            acc = acc + res.results[b * HPC + g]["yT"]
        out[b] = acc.T
    return out
